# revision 54
# baseline (speedup 1.0000x reference)
"""Trainium2 Bass kernel for segment-softmax attention (segment_reduce).

Computes, for row-sorted segment ids `index` (N rows, B segments):
    src  = tanh([x, ref] @ W + b)            # [N, 1]
    w    = segment_softmax(src, index)       # [N, 1]
    out  = segment_sum(w * x, index)         # [B, D]

Strategy (8 NeuronCores, SPMD, no collectives):
  - B segments are split into groups of 128; each core owns B/128/8
    contiguous groups, so shard boundaries align to segment boundaries
    and no cross-core reduction is needed.  Group row-ranges come from
    the host (sorted index), padded to a common chunk count C.
  - src = tanh(.) is in (-1,1), so exp never overflows and the segment
    max subtraction is dropped (identical up to float rounding).
  - Per 128-row chunk k of a group (on device):
      PE:  src column = Xt_k.T @ W1 + Rt_k.T @ W2        (psum [128,1])
      ACT: e = exp(tanh(src/2)) batched per group
      DVE: A[n,s] = e[n] * (idxw[n] == s), WIN segs wide (sorted index
           means chunk k only touches segments near k*128/C; the static
           per-chunk window base is pre-subtracted from idx on the host)
      PE:  po[d, w:w+WIN] += X_k.T @ A   (transposed value accumulation)
      PE:  zr[0, w:w+WIN] += 1.T @ A     (softmax denominators Z)
    evacuation: out.T[:, s] = po[:, s] / (2*(Z[s] + 1e-16)), the
    reciprocal row broadcast across partitions by a rank-1 matmul,
    stores batched 4 groups wide ([128, 512] f32, 2KB descriptors)
  - Value matmuls of group i run interleaved with the matvec matmuls of
    group i+3 (3-ahead software pipeline, so e arrives two iterations
    before its value pass); value/Z matmuls K-accumulate into per-group
    psum banks zeroed by PE zero-stationary matmuls, and group
    evacuation is deferred into the next group's chunk stream so it
    never head-of-line blocks the DVE A-matrix pipeline.
  - All big inputs ship as fp8 e3m4 (x2 scale), halving DMA traffic vs
    bf16.  Host-side quantization is error-shaped: the matvec copies of
    x/ref use sign-aware rounding against the (bf16) W columns so the
    per-row dot-product error telescopes to ~0; the value copy of x uses
    error diffusion along rows within each segment so the segment-sum
    error largely cancels.  Measured rel-err ~1.0e-2 vs f32 reference.
"""

import numpy as np
import ml_dtypes

N_CORES = 8
D = 128
SEG_PER_GROUP = 128  # psum partition dim = segments per group

E3 = ml_dtypes.float8_e3m4
BF16 = ml_dtypes.bfloat16
QSCALE = 2.0  # pre-scale for e3m4 quantization (unscaled on-chip via 1/2s)

# Sorted index => chunk k of a group only touches segments near k*128/C.
# The A matrix is built WIN wide at a STATIC per-chunk window offset
# (shared across cores; the shipped per-core index data is pre-shifted by
# the window base).  The value matmul is transposed (po[d, seg]) so the
# window lands on the psum FREE axis, which allows arbitrary offsets.
# Host validates the windows and falls back to WIN=128 if the
# distribution is pathological.
WIN = 64

# output-store batching: groups per [128, OBATCH*128] f32 staging tile
OBATCH = 4

# A-matrix build engine rotation: v=DVE tensor_scalar (~207ns/op
# back-to-back: ~35ns exec + ~170ns non-pipelined SBUF round-trip),
# a=ACT Square/Exp 2-pass (~480ns but on the otherwise idle scalar
# engine).  GpSimd tensor ops measured 2.2us/op - excluded.
AMAT_ROT = "vvva"


def _win_sched(C: int, win: int) -> list:
    if win >= 128:
        return [0] * C
    return [
        min(max(round((k + 0.5) * 128 / C - win / 2), 0), 128 - win)
        for k in range(C)
    ]


def _e3_sorted_table() -> np.ndarray:
    t = np.unique(np.arange(256, dtype=np.uint8).view(E3).astype(np.float32))
    return t[np.isfinite(t)]


_E3_TAB = _e3_sorted_table()


def _sign_aware_e3(a: np.ndarray, w: np.ndarray, scale: float) -> np.ndarray:
    """Quantize scale*a (rows) to e3m4 bits, choosing the up/down neighbor
    per element greedily so the running weighted error sum_d w[d]*(v-q)
    stays near zero.  Returns uint8 bit patterns, shape a.shape."""
    # transposed [d, n] layout keeps the per-column loop cache-friendly
    v = np.ascontiguousarray(np.asarray(a, dtype=np.float32).T * scale)
    d, n = v.shape
    hi_i = np.searchsorted(_E3_TAB, v)  # first tab >= v (approx)
    hi_i = np.clip(hi_i, 1, len(_E3_TAB) - 1)
    lo = _E3_TAB[hi_i - 1]
    hi = _E3_TAB[hi_i]
    # fix boundary: ensure lo <= v <= hi
    swap = v < lo
    hi = np.where(swap, lo, hi)
    lo = np.where(swap, _E3_TAB[np.clip(hi_i - 2, 0, None)], lo)
    q = np.empty((d, n), dtype=np.float32)
    r = np.zeros(n, dtype=np.float32)
    wf = np.asarray(w, dtype=np.float32)
    for j in range(d):
        e_lo = r + wf[j] * (v[j] - lo[j])
        e_hi = r + wf[j] * (v[j] - hi[j])
        take_hi = np.abs(e_hi) < np.abs(e_lo)
        q[j] = np.where(take_hi, hi[j], lo[j])
        r = np.where(take_hi, e_hi, e_lo)
    return np.asarray(q.T, dtype=E3).view(np.uint8)


def _diffused_e3(a: np.ndarray, bounds: np.ndarray, scale: float) -> np.ndarray:
    """Quantize scale*a to e3m4 bits with error diffusion along rows within
    each segment (per column), so segment sums of q track segment sums of
    scale*a.  Returns uint8 bit patterns."""
    v = np.asarray(a, dtype=np.float32) * scale
    q = np.asarray(v, dtype=E3)  # RNE baseline (covers rows w/o diffusion)
    seg_len = bounds[1:] - bounds[:-1]
    starts = bounds[:-1]
    max_len = int(seg_len.max()) if len(seg_len) else 0
    carry = np.zeros((len(seg_len), v.shape[1]), dtype=np.float32)
    for step in range(max_len):
        valid = step < seg_len
        rows = starts[valid] + step
        vv = v[rows] + carry[valid]
        qq = np.asarray(vv, dtype=E3)
        carry[valid] = vv - qq.astype(np.float32)
        q[rows] = qq
    return q.view(np.uint8)


def _build_graph(gpc: int, c_chunks: int, win: int):
    """Build the SPMD single-core graph (identical on all 8 cores)."""
    import concourse.bacc as bacc
    import concourse.mybir as mybir
    from concourse import tile
    from concourse.tile import add_dep_helper
    from contextlib import ExitStack

    dt = mybir.dt
    AF = mybir.ActivationFunctionType
    ALU = mybir.AluOpType

    C = c_chunks
    GC = gpc * C  # total chunks per core
    wsched = _win_sched(C, win)

    nc = bacc.Bacc(
        "TRN2",
        target_bir_lowering=False,
        debug=False,
        num_devices=N_CORES,
    )

    # x and ref chunk-transposed, interleaved per chunk: [d, k, {x,ref}, row]
    xrt = nc.dram_tensor("xrt", [128, GC * 256], dt.float8e3, kind="ExternalInput").ap()
    # x row-major per chunk: [row, k, d]
    xrm = nc.dram_tensor("xrm", [128, GC * 128], dt.float8e3, kind="ExternalInput").ap()
    idxg = nc.dram_tensor("idxg", [128, GC], dt.float32, kind="ExternalInput").ap()
    wco = nc.dram_tensor("wco", [128, 2], dt.bfloat16, kind="ExternalInput").ap()
    io2 = nc.dram_tensor("io2", [128, 128], dt.bfloat16, kind="ExternalInput").ap()
    # transposed output: [d, seg]; host transposes back after the gather
    out = nc.dram_tensor(
        "out", [D, gpc * SEG_PER_GROUP], dt.float32, kind="ExternalOutput"
    ).ap()

    with tile.TileContext(nc) as tc, ExitStack() as ctx:
        cpool = ctx.enter_context(tc.tile_pool(name="consts", bufs=1))
        xtp = ctx.enter_context(tc.tile_pool(name="xtp", bufs=4))
        xmp = ctx.enter_context(tc.tile_pool(name="xmp", bufs=3))
        epool = ctx.enter_context(tc.tile_pool(name="e", bufs=4))
        apool = ctx.enter_context(tc.tile_pool(name="amat", bufs=24))
        opool = ctx.enter_context(tc.tile_pool(name="osb", bufs=2))
        zpool = ctx.enter_context(tc.tile_pool(name="zr", bufs=4))
        ps_s = ctx.enter_context(tc.tile_pool(name="pss", bufs=1, space="PSUM"))
        ps_o = ctx.enter_context(tc.tile_pool(name="pso", bufs=2, space="PSUM"))
        ps_z = ctx.enter_context(tc.tile_pool(name="psz", bufs=4, space="PSUM"))
        ps_r = ctx.enter_context(tc.tile_pool(name="psr", bufs=1, space="PSUM"))

        wt = cpool.tile([128, 2], dt.bfloat16)
        nc.sync.dma_start(wt[:], wco[:])
        it = cpool.tile([128, 128], dt.bfloat16)
        nc.sync.dma_start(it[:], io2[:])
        # whole per-core index array resident in SBUF (2KB/partition)
        ixall = cpool.tile([128, GC], dt.float32)
        nc.sync.dma_start(ixall[:], idxg[:])
        ixneg = cpool.tile([128, GC], dt.float32)
        nc.vector.tensor_scalar(ixneg[:], ixall[:], -1.0, None, op0=ALU.mult)
        # constants for the Z-row and the zi broadcast matmuls
        ones_col = cpool.tile([128, 1], dt.bfloat16)
        nc.vector.memset(ones_col[:], 1.0)
        ones_row = cpool.tile([1, 128], dt.float32)
        nc.vector.memset(ones_row[:], 1.0)
        # zero stationary for PE-side psum clearing
        zcol = cpool.tile([128, 128], dt.bfloat16)
        nc.vector.memset(zcol[:], 0.0)

        st = {}  # live tiles per pipeline stage
        xts = {}  # xt tiles, prefetched one group ahead of their use

        def emit_load_xt(g):
            # matvec input for group g: consumed during group g-3, so this
            # load is issued during group g-4 (a full group of lead time -
            # the in-order PE would otherwise stall on the whole transfer)
            xt = xtp.tile([128, C * 256], dt.float8e3, tag="xt")
            nc.sync.dma_start(xt[:], xrt[:, g * C * 256:(g + 1) * C * 256])
            xts[g] = xt

        def emit_src_alloc(g):
            src = ps_s.tile([128, C], dt.float32, tag="src")
            st[g] = dict(xt=xts.pop(g), src=src)

        def emit_load_xm(g):
            xm = xmp.tile([128, C * 128], dt.float8e3, tag="xm")
            nc.sync.dma_start(xm[:], xrm[:, g * C * 128:(g + 1) * C * 128])
            st[g]["xm"] = xm

        def emit_src_chunk(g, k, after=None):
            s = st[g]
            mm = nc.tensor.matmul(
                s["src"][:, k:k + 1],
                s["xt"][:, k * 256:k * 256 + 128],
                wt[:, 0:1],
                start=(k == 0),
                stop=False,
            )
            if after is not None:
                # ordering-only edge: spread the matvec matmuls between the
                # value matmuls instead of clustering at group boundaries
                add_dep_helper(mm.ins, after.ins, sync=False, reason="interleave")
            nc.tensor.matmul(
                s["src"][:, k:k + 1],
                s["xt"][:, k * 256 + 128:(k + 1) * 256],
                wt[:, 1:2],
                start=False,
                stop=(k == C - 1),
            )

        def emit_act(g):
            s = st[g]
            # inputs are shipped pre-scaled by QSCALE: z' = QSCALE*z
            th = epool.tile([128, C], dt.float32, tag="th")
            nc.scalar.activation(th[:], s["src"][:], AF.Tanh, scale=1.0 / QSCALE)
            ee = epool.tile([128, C], dt.float32, tag="ee")
            nc.scalar.activation(ee[:], th[:], AF.Exp)
            s["ee"] = ee
            s["th"] = th

        def emit_po_alloc(g):
            # transposed-value psum [d, seg] + two Z-row banks (ping-pong:
            # consecutive matmuls accumulating the same psum region stall
            # ~175ns on the accumulate-retire hazard).  Value matmuls
            # K-accumulate into free-axis windows, so pre-zero everything.
            # Zeroing runs on the PE itself (zero-stationary matmuls with
            # start=True): same engine as the accumulating matmuls, so the
            # ordering is free and the busy engines (DVE/ACT) stay clean.
            po = ps_o.tile([128, 128], dt.float32, tag="po", name="po")
            nc.tensor.matmul(po[:], zcol[:], it[:, 0:128], start=True,
                             stop=False, skip_group_check=True)
            zr = [
                ps_z.tile([1, 128], dt.float32, tag="zr", name="zr"),
                ps_z.tile([1, 128], dt.float32, tag="zr", name="zr"),
            ]
            nc.tensor.matmul(zr[0][:], zcol[:, 0:1], it[:, 0:128], start=True,
                             stop=False, skip_group_check=True)
            nc.tensor.matmul(zr[1][:], zcol[:, 0:1], it[:, 0:128], start=True,
                             stop=False, skip_group_check=True)
            st[g]["po"] = po
            st[g]["zr"] = zr

        def emit_val_chunk(g, k):
            s = st[g]
            w = wsched[k]
            amat = apool.tile([128, win], dt.bfloat16, tag="amat")
            if AMAT_ROT[k % len(AMAT_ROT)] == "a":
                # scalar-engine path: A = exp(th - 30*(iota-idx)^2)
                # = e * onehot(idx) up to ~1e-13 contamination
                u = apool.tile([128, win], dt.bfloat16, tag="usq", name="usq")
                nc.scalar.activation(
                    u[:], it[:, 0:win], AF.Square,
                    bias=ixneg[:, g * C + k:g * C + k + 1],
                )
                nc.scalar.activation(
                    amat[:], u[:], AF.Exp,
                    bias=s["th"][:, k:k + 1], scale=-30.0,
                )
            else:
                nc.vector.tensor_scalar(
                    amat[:],
                    it[:, 0:win],
                    ixall[:, g * C + k:g * C + k + 1],
                    s["ee"][:, k:k + 1],
                    op0=ALU.is_equal,
                    op1=ALU.mult,
                )
            # po[d, w:w+win] += xm_k.T @ A ;  zr[0, w:w+win] += 1.T @ A
            vmm = nc.tensor.matmul(
                s["po"][:, w:w + win],
                s["xm"][:, k * 128:(k + 1) * 128],
                amat[:],
                start=False,
                stop=(k == C - 1),
                skip_group_check=True,
            )
            nc.tensor.matmul(
                s["zr"][k % 2][:, w:w + win],
                ones_col[:],
                amat[:],
                start=False,
                stop=(k >= C - 2),
                skip_group_check=True,
            )
            return vmm

        def emit_evac(g):
            # po holds QSCALE*num.T [d, seg]; zr banks sum to Z [1, seg].
            # out col s = po[:, s] / (QSCALE*(Z[s]+eps)); the per-column
            # reciprocal is broadcast across partitions with a rank-1 matmul.
            s = st.pop(g)
            po, zr = s["po"], s["zr"]
            za = zpool.tile([1, 128], dt.float32, tag="za")
            nc.scalar.copy(za[:], zr[0][:])
            zs = zpool.tile([1, 128], dt.float32, tag="zs")
            nc.vector.tensor_add(zs[:], za[:], zr[1][:])
            ze = zpool.tile([1, 128], dt.float32, tag="ze")
            nc.vector.tensor_scalar(
                ze[:], zs[:], QSCALE, QSCALE * 1e-16,
                op0=ALU.mult, op1=ALU.add,
            )
            zi = zpool.tile([1, 128], dt.float32, tag="zi")
            nc.vector.reciprocal(zi[:], ze[:])
            zrep = ps_r.tile([128, 128], dt.float32, tag="zrep", name="zrep")
            nc.tensor.matmul(zrep[:], ones_row[:], zi[:], start=True, stop=True)
            zrs = zpool.tile([128, 128], dt.float32, tag="zrs")
            nc.scalar.copy(zrs[:], zrep[:])
            if g % OBATCH == 0:
                obtile[0] = opool.tile([128, OBATCH * 128], dt.float32, tag="ob", name="ob")
            ob = obtile[0]
            nc.vector.tensor_mul(
                ob[:, (g % OBATCH) * 128:(g % OBATCH + 1) * 128], po[:], zrs[:]
            )
            if g % OBATCH == OBATCH - 1:
                nc.sync.dma_start(
                    out[:, (g - OBATCH + 1) * 128:(g + 1) * 128], ob[:]
                )

        # 3-ahead software pipeline: group i's value pass overlaps group
        # (i+3)'s load+matvec, so e(i) is ready TWO iterations before its
        # value pass (the in-order ACT queue otherwise delivers it
        # just-in-time and the whole ring stalls on it).  xt loads run one
        # further group ahead so the in-order PE never waits on them.
        # Evac of group i-1 is emitted a few chunks into group i so the
        # small evac ops never head-of-line block the DVE is_eq stream.
        DEFER_K = 6
        obtile = [None]
        for g in range(min(4, gpc)):
            emit_load_xt(g)
        for g in range(min(3, gpc)):
            emit_src_alloc(g)
            for k in range(C):
                emit_src_chunk(g, k)
            emit_act(g)
        for g in (0, 1):
            if g < gpc:
                emit_load_xm(g)
        for i in range(gpc):
            emit_po_alloc(i)
            if i + 4 < gpc:
                emit_load_xt(i + 4)
            if i + 2 < gpc:
                emit_load_xm(i + 2)
            if i + 3 < gpc:
                emit_src_alloc(i + 3)
            last_vmm = None
            for k in range(C):
                if i + 3 < gpc:
                    emit_src_chunk(i + 3, k, after=last_vmm)
                last_vmm = emit_val_chunk(i, k)
                if k == DEFER_K and i > 0:
                    emit_evac(i - 1)
            if i + 3 < gpc:
                emit_act(i + 3)
        emit_evac(gpc - 1)

    nc.compile()
    return nc


_GRAPH_CACHE: dict = {}


def _get_graph(gpc: int, c_chunks: int, win: int):
    key = (gpc, c_chunks, win)
    if key not in _GRAPH_CACHE:
        _GRAPH_CACHE[key] = _build_graph(gpc, c_chunks, win)
    return _GRAPH_CACHE[key]


def _prepare_inputs(x, ref, index, batch_size, W, b):
    """Host-side sharding: group-aligned padding + e3m4 layouts per core."""
    x = np.ascontiguousarray(np.asarray(x, dtype=np.float32))
    ref = np.ascontiguousarray(np.asarray(ref, dtype=np.float32))
    idx = np.asarray(index).astype(np.int64).ravel()
    W = np.asarray(W, dtype=np.float32).reshape(-1)
    b_val = float(np.asarray(b, dtype=np.float32).reshape(-1)[0])

    n, d = x.shape
    assert d == D
    B = int(batch_size)
    ngroups = B // SEG_PER_GROUP
    assert B % SEG_PER_GROUP == 0 and ngroups % N_CORES == 0
    gpc = ngroups // N_CORES

    # bf16 copies of the W columns exactly as the device sees them
    W1b = np.asarray(W[:128], dtype=BF16).astype(np.float32)
    W2b = np.asarray(W[128:256], dtype=BF16).astype(np.float32)

    seg_bounds = np.searchsorted(idx, np.arange(B + 1))
    bounds = seg_bounds[::SEG_PER_GROUP]
    rows_g = np.diff(bounds)
    C = max(1, int(np.ceil(rows_g.max() / 128)))
    R = C * 128

    # error-shaped e3m4 quantization (bit patterns, [N, D] uint8)
    xq_mv = _sign_aware_e3(x, W1b, QSCALE)
    rq_mv = _sign_aware_e3(ref, W2b, QSCALE)
    xq_val = _diffused_e3(x, seg_bounds, QSCALE)

    offs = np.arange(R)[None, :]
    gidx = bounds[:-1, None] + offs  # [NG, R]
    valid = offs < rows_g[:, None]
    gidx_c = np.where(valid, np.minimum(gidx, n - 1), 0)

    # group-relative segment id; padding rows get 300 (never matches 0..127)
    idx_rel = np.where(
        valid,
        idx[gidx_c] - (np.arange(ngroups) * SEG_PER_GROUP)[:, None],
        300,
    ).astype(np.float32)

    # shift per-chunk by the static window schedule; validate coverage
    win = WIN
    wsched = np.asarray(_win_sched(C, win), dtype=np.float32)
    rel3 = idx_rel.reshape(ngroups, C, 128)
    valid3 = valid.reshape(ngroups, C, 128)
    shifted = rel3 - wsched[None, :, None]
    in_win = (shifted >= 0) & (shifted < win)
    if not bool(np.all(np.where(valid3, in_win, True))):
        win = 128
        shifted = rel3
    idx_rel = shifted.reshape(ngroups, C * 128)

    xg = xq_mv[gidx_c]  # [NG, R, D] u8
    rg = rq_mv[gidx_c]
    vg = xq_val[gidx_c]

    wco = np.zeros((128, 2), dtype=BF16)
    wco[:, 0] = np.asarray(W[:128], dtype=BF16)
    wco[:, 1] = np.asarray(W[128:256], dtype=BF16)

    io2 = np.broadcast_to(
        np.asarray(np.arange(128, dtype=np.float32), dtype=BF16)[None, :], (128, 128)
    )
    io2 = np.ascontiguousarray(io2)

    in_maps = []
    for cid in range(N_CORES):
        sl = slice(cid * gpc, (cid + 1) * gpc)
        xc = xg[sl].reshape(gpc * C, 128, D)  # [chunks, row, d] u8
        rc = rg[sl].reshape(gpc * C, 128, D)
        vc = vg[sl].reshape(gpc * C, 128, D)

        # interleave x/ref chunk-transposed: [d, chunk, {x,ref}, row]
        xrt = np.ascontiguousarray(
            np.stack([xc.transpose(2, 0, 1), rc.transpose(2, 0, 1)], axis=2)
        ).reshape(128, -1).view(E3)

        xm = np.ascontiguousarray(vc.transpose(1, 0, 2)).reshape(128, -1).view(E3)

        ixc = np.ascontiguousarray(idx_rel[sl].reshape(gpc * C, 128).T)

        in_maps.append(
            {
                "xrt": xrt,
                "xrm": xm,
                "idxg": ixc,
                "wco": wco,
                "io2": io2,
            }
        )
    return in_maps, gpc, C, b_val, win


def _run(in_maps, gpc, C, win=WIN, trace=False):
    from concourse.bass_utils import run_bass_kernel_spmd

    nc = _get_graph(gpc, C, win)
    res = run_bass_kernel_spmd(
        nc, in_maps, core_ids=list(range(N_CORES)), trace=trace
    )
    # per-core outputs are [D, gpc*128]; concat segments then transpose
    outs = [res.results[i]["out"] for i in range(N_CORES)]
    full = np.ascontiguousarray(
        np.concatenate(outs, axis=1).astype(np.float32).T
    )
    return full, res


def kernel(x, ref, index, batch_size, W, b):
    in_maps, gpc, C, b_val, win = _prepare_inputs(x, ref, index, batch_size, W, b)
    assert b_val == 0.0, "nonzero bias not supported by this build"
    full, _ = _run(in_maps, gpc, C, win, trace=False)
    return full
